# revision 19
# baseline (speedup 1.0000x reference)
"""Trainium2 Bass kernel for CustomRBF forward:

    out[i] = w * exp(-gamma * ||X[i] - centroid||^2) + b

Flat-contiguous layout (per core, data-parallel over 8 cores):
  - The 125056-sample shard is viewed flat as [128 partitions, 977 samples
    * 128 feats]: partition p holds samples [p*977, (p+1)*977) of the
    shard, each partition line a fully CONTIGUOUS 500,224-byte DRAM run.
    DMA chunks of `ch` sample-columns load [128, ch*128] tiles whose
    per-partition reads are ch*512 contiguous bytes. Input DMA is the
    roofline: 64MB/core at ~364 GB/s/core (2.9 TB/s chip HBM / 8) =
    ~176 us; everything else must hide under it.

v2 structure — three work paths, assigned per half-group (hg=8 sample
columns), each a multi-engine pipeline with 1-hg software skew so every
engine's strict-FIFO queue only ever holds work whose inputs were
emitted >=1 hg earlier:
  - A-path (every pe_every'th hg): TensorE transpose per 128-sample
    column slice -> PSUM [feat, p]; ScalarE fused subtract+square
    (Square, bias=-c); TensorE f32r matmul (y stationary, ones moving)
    reduces feature partitions -> PSUM acc column (lands directly in
    [partition, sample-col] output layout).  ~2.9us PE per hg.
  - Bg-path: GPSIMD tensor_sub (natural layout, x - crep) -> ScalarE
    Square -> VectorE segmented tensor_reduce into the same acc.
    GPSIMD elementwise runs at ~0.42 eff (~2.0us/hg); ACT ~1.0us; DVE
    reduce ~1.2us.
  - Bv-path: same but the sub on VectorE (~1.2us/hg).
  Mix (pe_every=3, gpsimd 3 of 5 B-subs) puts every engine at 100-140us
  total busy, under the 176us DMA roofline.
  - Finalize is CHUNKED (fin_chunks): as soon as all hgs covering a
    column range have reduced, ScalarE Exp (scale=-gamma) acc->SBUF,
    VectorE tensor_scalar (*w + b), output DMA on gpsimd/SWDGE. Only the
    last chunk trails the last DMA.
  - DMA schedule: small first chunk (fast pipeline fill), big middle
    chunks, small last chunk (short tail).

Sharding: cores 0-6 take contiguous 125056-sample slices; core 7 takes the
last 125056 samples (overlapping core 6 by 448 samples; the overlap is
recomputed identically and overwritten at gather time).

`repeats` re-emits the whole pipeline R times in one NEFF (same data, same
output) -- used only for differential wall-clock timing of the steady state.
"""

import sys

sys.path.insert(0, "/opt/trn_rl_repo")

import numpy as np

D = 128          # feature dim
P = 128          # SBUF partitions
GAMMA = 1.0 / D
N_CORES = 8
SPP = 977        # samples per partition
SHARD = P * SPP  # 125056
N_TOTAL = 1000000
HG = 16          # max sample-columns per half-group (const sizing)

_NC_CACHE = {}

# kernel() build config (current best known)
BEST = dict(ch=56, ch_first=16, ch_last=16, xin_bufs=4, a_num=1, a_den=3,
            gs_num=1, gs_den=1, fin_chunks=4)


def _build(spp=SPP, repeats=1, hg=8, ch=56, ch_first=16, ch_last=16,
           xin_bufs=4, df_bufs=3, sq_bufs=3, y_bufs=4, tr_bufs=2,
           pe_every=3, a_num=None, a_den=None, gs_num=3, gs_den=5,
           fin_chunks=4, nmov=2,
           s2lag=1, s3lag=1, sq_eng="act", tmpl=None, tail_v=1,
           tail_path="V", taper=(), xdt="f32", stage="full",
           dma_eng="sync", out_eng="gpsimd", ydt="f32r"):
    from contextlib import ExitStack

    import concourse.tile as tile
    from concourse import bacc, mybir

    f32 = mybir.dt.float32
    ydtype = {"f32r": mybir.dt.float32r, "f32": f32,
              "bf16": mybir.dt.bfloat16}[ydt]
    xdtype = {"f32": f32, "bf16": mybir.dt.bfloat16}[xdt]
    Act = mybir.ActivationFunctionType
    Alu = mybir.AluOpType

    nc = bacc.Bacc("TRN2", target_bir_lowering=False, debug=False,
                   num_devices=N_CORES)
    xh = nc.declare_dram_parameter("x", [P, spp * D], f32, isOutput=False)
    negch = nc.declare_dram_parameter("negc", [P, 1], f32, isOutput=False)
    identh = nc.declare_dram_parameter("ident", [P, D], f32, isOutput=False)
    onesh = nc.declare_dram_parameter("ones", [P, 2], f32, isOutput=False)
    creph = nc.declare_dram_parameter("crep", [P, HG * D], f32,
                                      isOutput=False)
    wh = nc.declare_dram_parameter("wvec", [P, 1], f32, isOutput=False)
    bh = nc.declare_dram_parameter("bvec", [P, 1], f32, isOutput=False)
    outh = nc.declare_dram_parameter("out", [P, spp], f32, isOutput=True)

    # DMA chunk schedule (in sample-columns, multiples of hg except last)
    sched = []
    rem = spp
    if ch_first and rem > ch_first:
        sched.append(ch_first)
        rem -= ch_first
    tail_sched = [t for t in taper]
    if ch_last:
        tail_sched.append(ch_last)
    tail_total = sum(tail_sched)
    if rem > tail_total:
        rem -= tail_total
    else:
        tail_sched = []
    while rem > 0:
        c = min(ch, rem)
        sched.append(c)
        rem -= c
    sched.extend(tail_sched)
    assert sum(sched) == spp
    ch_max = max(sched)

    with ExitStack() as ctx:
        tc = ctx.enter_context(tile.TileContext(nc))
        singles = ctx.enter_context(tc.tile_pool(name="singles", bufs=1))
        xin = ctx.enter_context(tc.tile_pool(name="xin", bufs=xin_bufs))
        dfp = ctx.enter_context(tc.tile_pool(name="df", bufs=df_bufs))
        sqp = ctx.enter_context(tc.tile_pool(name="sq", bufs=sq_bufs))
        yp = ctx.enter_context(tc.tile_pool(name="y", bufs=y_bufs))
        resp = ctx.enter_context(tc.tile_pool(name="res", bufs=2))
        trp = ctx.enter_context(tc.tile_pool(name="tr", bufs=tr_bufs,
                                             space="PSUM"))
        acc_space = "PSUM" if xdtype is f32 else "SBUF"
        accp = ctx.enter_context(tc.tile_pool(name="acc", bufs=1,
                                              space=acc_space))

        negc_s = singles.tile([P, 1], f32)
        nc.sync.dma_start(out=negc_s, in_=negch[:, :])
        ident_s = singles.tile([P, D], f32)
        nc.sync.dma_start(out=ident_s, in_=identh[:, :])
        ones_s = singles.tile([P, 2], f32)
        nc.sync.dma_start(out=ones_s, in_=onesh[:, :])
        ones_r = singles.tile([P, 2], ydtype)
        nc.vector.tensor_copy(out=ones_r, in_=ones_s)
        crep_s = singles.tile([P, HG * D], f32)
        nc.sync.dma_start(out=crep_s, in_=creph[:, :])
        if xdtype is not f32:
            crep_x = singles.tile([P, HG * D], xdtype)
            nc.vector.tensor_copy(out=crep_x, in_=crep_s)
        else:
            crep_x = crep_s
        crep3 = crep_x.rearrange("p (t k) -> p t k", k=D)
        wv_s = singles.tile([P, 1], f32)
        nc.sync.dma_start(out=wv_s, in_=wh[:, :])
        bv_s = singles.tile([P, 1], f32)
        nc.sync.dma_start(out=bv_s, in_=bh[:, :])

        for _rep in range(repeats):
            acc = accp.tile([P, spp * nmov], f32, name="acc", tag="acc")

            def acc_view(c0, n):
                return acc.rearrange("p (t two) -> p t two",
                                     two=nmov)[:, c0:c0 + n, 0:1]

            # ---- per-hg stage closures, emitted with software skew ----
            def mk_a(xt3, h, hw, col):
                tr = trp.tile([P, 8 * D], f32, name="tr", tag="tr")
                y = yp.tile([P, 8 * D], ydtype, name="y", tag="y")

                def s1():
                    if stage == "nosub":
                        return
                    for j in range(hw):
                        nc.tensor.transpose(out=tr[:, j * D:(j + 1) * D],
                                            in_=xt3[:, h + j, :],
                                            identity=ident_s[:, :])

                def s2():
                    src_ap = (xt3[:, h:h + hw, :] if stage == "nosub"
                              else tr[:, :hw * D])
                    nc.scalar.activation(out=y[:, :hw * D],
                                         in_=src_ap,
                                         func=Act.Square,
                                         bias=negc_s[:, :], scale=1.0)

                def s3():
                    if stage == "nored":
                        return
                    for j in range(hw):
                        c = (col + j) * nmov
                        nc.tensor.matmul(out=acc[:, c:c + nmov],
                                         lhsT=y[:, j * D:(j + 1) * D],
                                         rhs=ones_r[:, :nmov],
                                         start=True, stop=True)

                return s1, s2, s3

            def mk_b(xt3, h, hw, col, sub_eng, bw=8):
                df = dfp.tile([P, bw, D], xdtype, name="df", tag="df")
                sq = sqp.tile([P, bw, D], xdtype, name="sq", tag="sq")

                def s1():
                    if stage == "nosub":
                        return
                    sub_eng.tensor_sub(out=df[:, :hw, :],
                                       in0=xt3[:, h:h + hw, :],
                                       in1=crep3[:, :hw, :])

                def s2():
                    if stage == "nosq":
                        return
                    src_ap = (xt3[:, h:h + hw, :] if stage == "nosub"
                              else df[:, :hw, :])
                    if sq_eng == "act":
                        nc.scalar.activation(out=sq[:, :hw, :],
                                             in_=src_ap,
                                             func=Act.Square, bias=0.0,
                                             scale=1.0)
                    else:
                        eng = nc.vector if sq_eng == "dve" else nc.gpsimd
                        eng.tensor_mul(out=sq[:, :hw, :], in0=src_ap,
                                       in1=src_ap)

                def s3():
                    if stage == "nored":
                        return
                    src_ap = (df[:, :hw, :] if stage == "nosq"
                              else sq[:, :hw, :])
                    nc.vector.tensor_reduce(out=acc_view(col, hw),
                                            in_=src_ap,
                                            axis=mybir.AxisListType.X,
                                            op=Alu.add)

                return s1, s2, s3

            # chunked finalize bookkeeping
            fin_w = -(-spp // fin_chunks)
            fin_next = [0]

            def maybe_finalize(done_cols, force=False):
                while (fin_next[0] < spp
                       and (done_cols - fin_next[0] >= fin_w
                            or (force and done_cols > fin_next[0]))):
                    c0 = fin_next[0]
                    w = min(fin_w, done_cols - c0, spp - c0)
                    res = resp.tile([P, fin_w], f32, name="res", tag="res")
                    nc.scalar.activation(out=res[:, :w],
                                         in_=acc_view(c0, w),
                                         func=Act.Exp, scale=-GAMMA,
                                         bias=0.0)
                    nc.vector.tensor_scalar(out=res[:, :w], in0=res[:, :w],
                                            scalar1=wv_s[:, :],
                                            scalar2=bv_s[:, :],
                                            op0=Alu.mult, op1=Alu.add)
                    oeng = {"sync": nc.sync, "scalar": nc.scalar,
                            "gpsimd": nc.gpsimd}[out_eng]
                    oeng.dma_start(out=outh[:, c0:c0 + w], in_=res[:, :w])
                    fin_next[0] += w

            # ---- main loop: DMA chunks -> skewed hg pipeline ----
            lag1 = []   # pending s2
            lag2 = []   # pending (s3, end_col)
            an, ad = (a_num, a_den) if a_num is not None else (
                (1, pe_every) if pe_every else (0, 1))
            tlist = None
            if tmpl:
                tlist = []
                for ent in tmpl.split(","):
                    tlist.append((ent[0], int(ent[1:])))
                assert all(p in "AGV" and (p != "A" or w <= 8)
                           and w <= HG for p, w in tlist)
            tcur = 0
            hg_idx = 0
            b_idx = 0
            col = 0
            for ci, cw in enumerate(sched):
                is_tail_chunk = ci >= len(sched) - tail_v
                xt = xin.tile([P, ch_max * D], xdtype, name="xt", tag="xt")
                if stage == "nodma":
                    nc.gpsimd.dma_start(out=xt[:, 0:D], in_=xh[:, 0:D])
                else:
                    eng = {"sync": nc.sync, "scalar": nc.scalar,
                           "gpsimd": nc.gpsimd}[dma_eng]
                    if xdtype is not f32:
                        eng = nc.gpsimd
                    eng.dma_start(out=xt[:, :cw * D],
                                  in_=xh[:, col * D:(col + cw) * D])
                if stage == "dma":
                    col += cw
                    continue
                xt3 = xt.rearrange("p (s k) -> p s k", k=D)
                h = 0
                while h < cw:
                    if is_tail_chunk:
                        hw = min(8, cw - h)
                        if tail_path == "A":
                            s1, s2, s3 = mk_a(xt3, h, hw, col + h)
                        else:
                            s1, s2, s3 = mk_b(xt3, h, hw, col + h,
                                              nc.vector)
                    elif tlist is not None:
                        path, w = tlist[tcur % len(tlist)]
                        tcur += 1
                        hw = min(w, cw - h)
                        if path == "A":
                            s1, s2, s3 = mk_a(xt3, h, hw, col + h)
                        else:
                            s1, s2, s3 = mk_b(
                                xt3, h, hw, col + h,
                                nc.gpsimd if path == "G" else nc.vector,
                                bw=(8 if w <= 8 else 16))
                    else:
                        hw = min(hg, cw - h)
                        is_a = ((hg_idx + 1) * an) // ad \
                            > (hg_idx * an) // ad
                        if is_a:
                            s1, s2, s3 = mk_a(xt3, h, hw, col + h)
                        else:
                            on_g = ((b_idx + 1) * gs_num) // gs_den \
                                > (b_idx * gs_num) // gs_den
                            s1, s2, s3 = mk_b(
                                xt3, h, hw, col + h,
                                nc.gpsimd if on_g else nc.vector)
                            b_idx += 1
                    s1()
                    lag1.append(s2)
                    if len(lag1) > s2lag:
                        lag1.pop(0)()
                    lag2.append((s3, col + h + hw))
                    if len(lag2) > s2lag + s3lag:
                        f3, ec = lag2.pop(0)
                        f3()
                        if stage not in ("dma", "nofin", "nored"):
                            maybe_finalize(ec)
                    hg_idx += 1
                    h += hw
                col += cw
            # drain the skew pipeline
            for f in lag1:
                f()
            for f3, ec in lag2:
                f3()
                if stage not in ("dma", "nofin", "nored"):
                    maybe_finalize(ec)
            if stage not in ("dma", "nofin", "nored"):
                maybe_finalize(spp, force=True)
        if stage in ("dma", "nofin", "nored"):
            nc.sync.dma_start(out=outh[:, 0:D], in_=ident_s[:, :])

    nc.finalize()
    return nc


def _get_nc():
    if "nc" not in _NC_CACHE:
        _NC_CACHE["nc"] = _build(**BEST)
    return _NC_CACHE["nc"]


def _make_const_inputs(centroid, w, b):
    centroid = np.asarray(centroid, dtype=np.float32).reshape(D)
    w = np.asarray(w, dtype=np.float32).reshape(-1)[0]
    b = np.asarray(b, dtype=np.float32).reshape(-1)[0]
    return {
        "negc": (-centroid).reshape(P, 1).copy(),
        "ident": np.eye(P, dtype=np.float32),
        "ones": np.tile(np.array([1.0, 0.0], dtype=np.float32), (P, 1)),
        "crep": np.tile(np.tile(centroid, HG), (P, 1)),
        "wvec": np.full((P, 1), w, dtype=np.float32),
        "bvec": np.full((P, 1), b, dtype=np.float32),
    }


def _shard_x(x_shard):
    # [SHARD, D] sample-major -> flat [P, SPP*D]: partition p holds
    # samples [p*SPP, (p+1)*SPP) as one contiguous run (pure view).
    return np.ascontiguousarray(x_shard).reshape(P, SPP * D)


def kernel(X, centroid, w, b, _trace=False, _trace_kwargs=None):
    from concourse.bass_utils import run_bass_kernel_spmd

    X = np.asarray(X)
    assert X.shape == (N_TOTAL, D), X.shape
    if X.dtype != np.float32:
        X = X.astype(np.float32)

    consts = _make_const_inputs(centroid, w, b)
    starts = [i * SHARD for i in range(N_CORES - 1)] + [N_TOTAL - SHARD]
    in_maps = [dict(consts, x=_shard_x(X[s:s + SHARD])) for s in starts]

    nc = _get_nc()
    kw = {}
    if _trace:
        kw = dict(trace=True, **(_trace_kwargs or {}))
    res = run_bass_kernel_spmd(nc, in_maps, list(range(N_CORES)), **kw)

    out = np.empty(N_TOTAL, dtype=np.float32)
    for i, s in enumerate(starts):
        out[s:s + SHARD] = res.results[i]["out"].reshape(-1)
    if _trace:
        return out, res
    return out


# revision 21
# speedup vs baseline: 1.0113x; 1.0113x over previous
"""Trainium2 Bass kernel for CustomRBF forward:

    out[i] = w * exp(-gamma * ||X[i] - centroid||^2) + b

Flat-contiguous layout (per core, data-parallel over 8 cores):
  - The 125056-sample shard is viewed flat as [128 partitions, 977 samples
    * 128 feats]: partition p holds samples [p*977, (p+1)*977) of the
    shard, each partition line a fully CONTIGUOUS 500,224-byte DRAM run.
    DMA chunks of `ch` sample-columns load [128, ch*128] tiles whose
    per-partition reads are ch*512 contiguous bytes (sync/HWDGE queue).

Two work paths, assigned per half-group (hg=8 sample columns) with a
1-hg software-pipeline skew (s2lag/s3lag) so each engine strict-FIFO
queue only holds work whose inputs were emitted earlier:
  - A-path (a_num/a_den = 1/3 of hgs): TensorE transpose per 128-sample
    column slice -> PSUM [feat, p]; ScalarE fused subtract+square
    (Square, bias=-c, per-partition = per-feature); TensorE f32r matmul
    (y stationary, ones moving) reduces feature partitions -> PSUM acc
    column (lands directly in [partition, sample-col] output layout).
    HW cost ~2.9us/hg on PE (transpose ~275ns/slice dominates).
  - B-path (rest): GPSIMD tensor_sub (gs_num/gs_den=1/1: all B-subs on
    GPSIMD, ~2.0us/hg at 0.42 elementwise eff) -> ScalarE Square
    (~1.0us/hg) -> VectorE segmented tensor_reduce into acc
    (~1.2us/hg).  DVE@0.96GHz is ~1 elem/cycle/partition for fp32;
    keeping DVE to just the reduce is what balances the engines
    (measured: all-DVE B-chain 321us, DVE-sub-heavy mixes 236-278us,
    this mix ~210us).
  - Finalize is CHUNKED (fin_chunks=4): as soon as all hgs covering a
    column range have reduced, ScalarE Exp (scale=-gamma) acc->SBUF,
    VectorE tensor_scalar (*w + b), output DMA on the scalar HWDGE queue
    (out_eng=scalar: keeps descriptor-gen off the loaded GPSIMD engine).
  - DMA schedule: small first chunk (16 cols) for fast pipeline fill,
    56-col middle chunks, small last chunk; the last `tail_v` chunks
    run on the V path (DVE sub) to shorten the post-DMA drain.

Rejected experimentally: bf16 compute pipeline via casting gpsimd DMA
(GPSIMD/DMA slower, 250-305us), 16-col B-groups (254us), deeper skew
(no effect), xdt=f32r transposes (254us), finer/other mixes.

Sharding: cores 0-6 take contiguous 125056-sample slices; core 7 takes the
last 125056 samples (overlapping core 6 by 448 samples; the overlap is
recomputed identically and overwritten at gather time).

`repeats` re-emits the whole pipeline R times in one NEFF (same data, same
output) -- used only for differential wall-clock timing of the steady state.
"""

import sys

sys.path.insert(0, "/opt/trn_rl_repo")

import numpy as np

D = 128          # feature dim
P = 128          # SBUF partitions
GAMMA = 1.0 / D
N_CORES = 8
SPP = 977        # samples per partition
SHARD = P * SPP  # 125056
N_TOTAL = 1000000
HG = 16          # max sample-columns per half-group (const sizing)

_NC_CACHE = {}

# kernel() build config (current best known)
BEST = dict(ch=56, ch_first=16, ch_last=16, xin_bufs=4, a_num=1, a_den=3,
            gs_num=1, gs_den=1, fin_chunks=4, out_eng="scalar")


def _build(spp=SPP, repeats=1, hg=8, ch=56, ch_first=16, ch_last=16,
           xin_bufs=4, df_bufs=3, sq_bufs=3, y_bufs=4, tr_bufs=2,
           pe_every=3, a_num=None, a_den=None, gs_num=3, gs_den=5,
           fin_chunks=4, nmov=2,
           s2lag=1, s3lag=1, sq_eng="act", tmpl=None, tail_v=1,
           tail_path="V", taper=(), xdt="f32", stage="full",
           dma_eng="sync", out_eng="gpsimd", ydt="f32r"):
    from contextlib import ExitStack

    import concourse.tile as tile
    from concourse import bacc, mybir

    f32 = mybir.dt.float32
    ydtype = {"f32r": mybir.dt.float32r, "f32": f32,
              "bf16": mybir.dt.bfloat16}[ydt]
    xdtype = {"f32": f32, "bf16": mybir.dt.bfloat16}[xdt]
    Act = mybir.ActivationFunctionType
    Alu = mybir.AluOpType

    nc = bacc.Bacc("TRN2", target_bir_lowering=False, debug=False,
                   num_devices=N_CORES)
    xh = nc.declare_dram_parameter("x", [P, spp * D], f32, isOutput=False)
    negch = nc.declare_dram_parameter("negc", [P, 1], f32, isOutput=False)
    identh = nc.declare_dram_parameter("ident", [P, D], f32, isOutput=False)
    onesh = nc.declare_dram_parameter("ones", [P, 2], f32, isOutput=False)
    creph = nc.declare_dram_parameter("crep", [P, HG * D], f32,
                                      isOutput=False)
    wh = nc.declare_dram_parameter("wvec", [P, 1], f32, isOutput=False)
    bh = nc.declare_dram_parameter("bvec", [P, 1], f32, isOutput=False)
    outh = nc.declare_dram_parameter("out", [P, spp], f32, isOutput=True)

    # DMA chunk schedule (in sample-columns, multiples of hg except last)
    sched = []
    rem = spp
    if ch_first and rem > ch_first:
        sched.append(ch_first)
        rem -= ch_first
    tail_sched = [t for t in taper]
    if ch_last:
        tail_sched.append(ch_last)
    tail_total = sum(tail_sched)
    if rem > tail_total:
        rem -= tail_total
    else:
        tail_sched = []
    while rem > 0:
        c = min(ch, rem)
        sched.append(c)
        rem -= c
    sched.extend(tail_sched)
    assert sum(sched) == spp
    ch_max = max(sched)

    with ExitStack() as ctx:
        tc = ctx.enter_context(tile.TileContext(nc))
        singles = ctx.enter_context(tc.tile_pool(name="singles", bufs=1))
        xin = ctx.enter_context(tc.tile_pool(name="xin", bufs=xin_bufs))
        dfp = ctx.enter_context(tc.tile_pool(name="df", bufs=df_bufs))
        sqp = ctx.enter_context(tc.tile_pool(name="sq", bufs=sq_bufs))
        yp = ctx.enter_context(tc.tile_pool(name="y", bufs=y_bufs))
        resp = ctx.enter_context(tc.tile_pool(name="res", bufs=2))
        trp = ctx.enter_context(tc.tile_pool(name="tr", bufs=tr_bufs,
                                             space="PSUM"))
        acc_space = "PSUM" if xdtype is f32 else "SBUF"
        accp = ctx.enter_context(tc.tile_pool(name="acc", bufs=1,
                                              space=acc_space))

        negc_s = singles.tile([P, 1], f32)
        nc.sync.dma_start(out=negc_s, in_=negch[:, :])
        ident_s = singles.tile([P, D], f32)
        nc.sync.dma_start(out=ident_s, in_=identh[:, :])
        ones_s = singles.tile([P, 2], f32)
        nc.sync.dma_start(out=ones_s, in_=onesh[:, :])
        ones_r = singles.tile([P, 2], ydtype)
        nc.vector.tensor_copy(out=ones_r, in_=ones_s)
        crep_s = singles.tile([P, HG * D], f32)
        nc.sync.dma_start(out=crep_s, in_=creph[:, :])
        if xdtype is not f32:
            crep_x = singles.tile([P, HG * D], xdtype)
            nc.vector.tensor_copy(out=crep_x, in_=crep_s)
        else:
            crep_x = crep_s
        crep3 = crep_x.rearrange("p (t k) -> p t k", k=D)
        wv_s = singles.tile([P, 1], f32)
        nc.sync.dma_start(out=wv_s, in_=wh[:, :])
        bv_s = singles.tile([P, 1], f32)
        nc.sync.dma_start(out=bv_s, in_=bh[:, :])

        for _rep in range(repeats):
            acc = accp.tile([P, spp * nmov], f32, name="acc", tag="acc")

            def acc_view(c0, n):
                return acc.rearrange("p (t two) -> p t two",
                                     two=nmov)[:, c0:c0 + n, 0:1]

            # ---- per-hg stage closures, emitted with software skew ----
            def mk_a(xt3, h, hw, col):
                tr = trp.tile([P, 8 * D], f32, name="tr", tag="tr")
                y = yp.tile([P, 8 * D], ydtype, name="y", tag="y")

                def s1():
                    if stage == "nosub":
                        return
                    for j in range(hw):
                        nc.tensor.transpose(out=tr[:, j * D:(j + 1) * D],
                                            in_=xt3[:, h + j, :],
                                            identity=ident_s[:, :])

                def s2():
                    src_ap = (xt3[:, h:h + hw, :] if stage == "nosub"
                              else tr[:, :hw * D])
                    nc.scalar.activation(out=y[:, :hw * D],
                                         in_=src_ap,
                                         func=Act.Square,
                                         bias=negc_s[:, :], scale=1.0)

                def s3():
                    if stage == "nored":
                        return
                    for j in range(hw):
                        c = (col + j) * nmov
                        nc.tensor.matmul(out=acc[:, c:c + nmov],
                                         lhsT=y[:, j * D:(j + 1) * D],
                                         rhs=ones_r[:, :nmov],
                                         start=True, stop=True)

                return s1, s2, s3

            def mk_b(xt3, h, hw, col, sub_eng, bw=8):
                df = dfp.tile([P, bw, D], xdtype, name="df", tag="df")
                sq = sqp.tile([P, bw, D], xdtype, name="sq", tag="sq")

                def s1():
                    if stage == "nosub":
                        return
                    sub_eng.tensor_sub(out=df[:, :hw, :],
                                       in0=xt3[:, h:h + hw, :],
                                       in1=crep3[:, :hw, :])

                def s2():
                    if stage == "nosq":
                        return
                    src_ap = (xt3[:, h:h + hw, :] if stage == "nosub"
                              else df[:, :hw, :])
                    if sq_eng == "act":
                        nc.scalar.activation(out=sq[:, :hw, :],
                                             in_=src_ap,
                                             func=Act.Square, bias=0.0,
                                             scale=1.0)
                    else:
                        eng = nc.vector if sq_eng == "dve" else nc.gpsimd
                        eng.tensor_mul(out=sq[:, :hw, :], in0=src_ap,
                                       in1=src_ap)

                def s3():
                    if stage == "nored":
                        return
                    src_ap = (df[:, :hw, :] if stage == "nosq"
                              else sq[:, :hw, :])
                    nc.vector.tensor_reduce(out=acc_view(col, hw),
                                            in_=src_ap,
                                            axis=mybir.AxisListType.X,
                                            op=Alu.add)

                return s1, s2, s3

            # chunked finalize bookkeeping
            fin_w = -(-spp // fin_chunks)
            fin_next = [0]

            def maybe_finalize(done_cols, force=False):
                while (fin_next[0] < spp
                       and (done_cols - fin_next[0] >= fin_w
                            or (force and done_cols > fin_next[0]))):
                    c0 = fin_next[0]
                    w = min(fin_w, done_cols - c0, spp - c0)
                    res = resp.tile([P, fin_w], f32, name="res", tag="res")
                    nc.scalar.activation(out=res[:, :w],
                                         in_=acc_view(c0, w),
                                         func=Act.Exp, scale=-GAMMA,
                                         bias=0.0)
                    nc.vector.tensor_scalar(out=res[:, :w], in0=res[:, :w],
                                            scalar1=wv_s[:, :],
                                            scalar2=bv_s[:, :],
                                            op0=Alu.mult, op1=Alu.add)
                    oeng = {"sync": nc.sync, "scalar": nc.scalar,
                            "gpsimd": nc.gpsimd}[out_eng]
                    oeng.dma_start(out=outh[:, c0:c0 + w], in_=res[:, :w])
                    fin_next[0] += w

            # ---- main loop: DMA chunks -> skewed hg pipeline ----
            lag1 = []   # pending s2
            lag2 = []   # pending (s3, end_col)
            an, ad = (a_num, a_den) if a_num is not None else (
                (1, pe_every) if pe_every else (0, 1))
            tlist = None
            if tmpl:
                tlist = []
                for ent in tmpl.split(","):
                    tlist.append((ent[0], int(ent[1:])))
                assert all(p in "AGV" and (p != "A" or w <= 8)
                           and w <= HG for p, w in tlist)
            tcur = 0
            hg_idx = 0
            b_idx = 0
            col = 0
            for ci, cw in enumerate(sched):
                is_tail_chunk = ci >= len(sched) - tail_v
                xt = xin.tile([P, ch_max * D], xdtype, name="xt", tag="xt")
                if stage == "nodma":
                    nc.gpsimd.dma_start(out=xt[:, 0:D], in_=xh[:, 0:D])
                else:
                    eng = {"sync": nc.sync, "scalar": nc.scalar,
                           "gpsimd": nc.gpsimd}[dma_eng]
                    if xdtype is not f32:
                        eng = nc.gpsimd
                    eng.dma_start(out=xt[:, :cw * D],
                                  in_=xh[:, col * D:(col + cw) * D])
                if stage == "dma":
                    col += cw
                    continue
                xt3 = xt.rearrange("p (s k) -> p s k", k=D)
                h = 0
                while h < cw:
                    if is_tail_chunk:
                        hw = min(8, cw - h)
                        if tail_path == "A":
                            s1, s2, s3 = mk_a(xt3, h, hw, col + h)
                        else:
                            s1, s2, s3 = mk_b(xt3, h, hw, col + h,
                                              nc.vector)
                    elif tlist is not None:
                        path, w = tlist[tcur % len(tlist)]
                        tcur += 1
                        hw = min(w, cw - h)
                        if path == "A":
                            s1, s2, s3 = mk_a(xt3, h, hw, col + h)
                        else:
                            s1, s2, s3 = mk_b(
                                xt3, h, hw, col + h,
                                nc.gpsimd if path == "G" else nc.vector,
                                bw=(8 if w <= 8 else 16))
                    else:
                        hw = min(hg, cw - h)
                        is_a = ((hg_idx + 1) * an) // ad \
                            > (hg_idx * an) // ad
                        if is_a:
                            s1, s2, s3 = mk_a(xt3, h, hw, col + h)
                        else:
                            on_g = ((b_idx + 1) * gs_num) // gs_den \
                                > (b_idx * gs_num) // gs_den
                            s1, s2, s3 = mk_b(
                                xt3, h, hw, col + h,
                                nc.gpsimd if on_g else nc.vector)
                            b_idx += 1
                    s1()
                    lag1.append(s2)
                    if len(lag1) > s2lag:
                        lag1.pop(0)()
                    lag2.append((s3, col + h + hw))
                    if len(lag2) > s2lag + s3lag:
                        f3, ec = lag2.pop(0)
                        f3()
                        if stage not in ("dma", "nofin", "nored"):
                            maybe_finalize(ec)
                    hg_idx += 1
                    h += hw
                col += cw
            # drain the skew pipeline
            for f in lag1:
                f()
            for f3, ec in lag2:
                f3()
                if stage not in ("dma", "nofin", "nored"):
                    maybe_finalize(ec)
            if stage not in ("dma", "nofin", "nored"):
                maybe_finalize(spp, force=True)
        if stage in ("dma", "nofin", "nored"):
            nc.sync.dma_start(out=outh[:, 0:D], in_=ident_s[:, :])

    nc.finalize()
    return nc


def _get_nc():
    if "nc" not in _NC_CACHE:
        _NC_CACHE["nc"] = _build(**BEST)
    return _NC_CACHE["nc"]


def _make_const_inputs(centroid, w, b):
    centroid = np.asarray(centroid, dtype=np.float32).reshape(D)
    w = np.asarray(w, dtype=np.float32).reshape(-1)[0]
    b = np.asarray(b, dtype=np.float32).reshape(-1)[0]
    return {
        "negc": (-centroid).reshape(P, 1).copy(),
        "ident": np.eye(P, dtype=np.float32),
        "ones": np.tile(np.array([1.0, 0.0], dtype=np.float32), (P, 1)),
        "crep": np.tile(np.tile(centroid, HG), (P, 1)),
        "wvec": np.full((P, 1), w, dtype=np.float32),
        "bvec": np.full((P, 1), b, dtype=np.float32),
    }


def _shard_x(x_shard):
    # [SHARD, D] sample-major -> flat [P, SPP*D]: partition p holds
    # samples [p*SPP, (p+1)*SPP) as one contiguous run (pure view).
    return np.ascontiguousarray(x_shard).reshape(P, SPP * D)


def kernel(X, centroid, w, b, _trace=False, _trace_kwargs=None):
    from concourse.bass_utils import run_bass_kernel_spmd

    X = np.asarray(X)
    assert X.shape == (N_TOTAL, D), X.shape
    if X.dtype != np.float32:
        X = X.astype(np.float32)

    consts = _make_const_inputs(centroid, w, b)
    starts = [i * SHARD for i in range(N_CORES - 1)] + [N_TOTAL - SHARD]
    in_maps = [dict(consts, x=_shard_x(X[s:s + SHARD])) for s in starts]

    nc = _get_nc()
    kw = {}
    if _trace:
        kw = dict(trace=True, **(_trace_kwargs or {}))
    res = run_bass_kernel_spmd(nc, in_maps, list(range(N_CORES)), **kw)

    out = np.empty(N_TOTAL, dtype=np.float32)
    for i, s in enumerate(starts):
        out[s:s + SHARD] = res.results[i]["out"].reshape(-1)
    if _trace:
        return out, res
    return out
